# revision 9
# baseline (speedup 1.0000x reference)
"""Trainium2 Bass kernel for nn_MetricLoss (pairwise-distance metric loss).

Computation (reference):
    f = x.reshape(1024, 49152)
    G = f @ f.T                      (103 GFLOP Gram matrix)
    dist = relu(sq_i + sq_j - 2 G)
    loss_homo  = 0.5 * sum(same-group dist)
    loss_heter = sum(cross-group relu(1 - dist))

Distribution (8 NeuronCores, one TRN2 chip):
    K-parallel: core c holds f[:, c*6144:(c+1)*6144].T as a [48, 128, 1024]
    bf16 tensor (k-major tiles). Each core computes a partial Gram
    [1024, 1024] over its K-slice via PSUM-accumulated matmuls (8 row-block
    chains x 4 column chunks of 256). Partial Grams are summed with 4
    chunked ReduceScatters so core c ends up with full-K Gram rows
    [128c:128c+128]. Row norms sq (= Gram diagonal) are extracted on-device
    with masked reduces and packed as 8 extra columns into the last RS so
    every core receives the full sq vector. A fused DVE epilogue computes
    the masked hinge sums; the host sums 8x[128,2] partials and normalizes.
"""

import numpy as np
import ml_dtypes

import concourse.bass as bass
import concourse.bacc as bacc
import concourse.tile as tile
import concourse.mybir as mybir
from concourse import bass_utils

F32 = mybir.dt.float32
BF16 = mybir.dt.bfloat16
ALU = mybir.AluOpType

N_CORES = 8
N = 1024            # batch (rows of f)
K = 64 * 768        # 49152 features per sample
KC = K // N_CORES   # 6144 features per core
KT = KC // 128      # 48 k-tiles of 128 per core
BK = 8              # samples per class group
NJ = 4              # column chunks
CW = N // NJ        # 256 columns per chunk
MB = N // 128       # 8 row blocks

_CACHE = {}


def _build_nc():
    nc = bacc.Bacc("TRN2", target_bir_lowering=False, debug=False,
                   num_devices=N_CORES)

    ft = nc.dram_tensor("ft", [KT, 128, N], BF16, kind="ExternalInput").ap()
    mask_same = nc.dram_tensor("mask_same", [128, N], F32,
                               kind="ExternalInput").ap()
    mask_diff = nc.dram_tensor("mask_diff", [128, N], F32,
                               kind="ExternalInput").ap()
    diagmask = nc.dram_tensor("diagmask", [128, 2, CW], F32,
                              kind="ExternalInput").ap()
    emask = nc.dram_tensor("emask", [128, 8], F32, kind="ExternalInput").ap()
    out = nc.dram_tensor("out", [128, 2], F32, kind="ExternalOutput").ap()

    rg = [list(range(N_CORES))]

    with tile.TileContext(nc) as tc:
        with (
            tc.tile_pool(name="ftp", bufs=1) as ftp,
            tc.tile_pool(name="misc", bufs=1) as misc,
            tc.tile_pool(name="gcopy", bufs=4) as gcp,
            tc.tile_pool(name="junk", bufs=2) as jkp,
            tc.tile_pool(name="psum", bufs=8, space="PSUM") as psp,
            tc.tile_pool(name="dram", bufs=1, space="DRAM") as drp,
        ):
            # ---- load inputs to SBUF ----
            ft_sb = []
            for k in range(KT):
                t = ftp.tile([128, N], BF16, tag=f"ft{k}", name=f"ft{k}")
                nc.sync.dma_start(t[:], ft[k])
                ft_sb.append(t)

            ms_sb = misc.tile([128, N], F32, tag="ms", name="ms")
            md_sb = misc.tile([128, N], F32, tag="md", name="md")
            dm_sb = misc.tile([128, 2, CW], F32, tag="dm", name="dm")
            em_sb = misc.tile([128, 8], F32, tag="em", name="em")
            nc.sync.dma_start(ms_sb[:], mask_same[:])
            nc.sync.dma_start(md_sb[:], mask_diff[:])
            nc.sync.dma_start(dm_sb[:], diagmask[:])
            nc.sync.dma_start(em_sb[:], emask[:])

            # sq partials per row block: sqp[:, b] = diag of block b
            sqp = misc.tile([128, 8], F32, tag="sqp", name="sqp")

            # ---- partial Gram: 4 column chunks x 8 row-block chains ----
            bounce = []
            for jc in range(NJ):
                w = CW + 8 if jc == NJ - 1 else CW
                bounce.append(drp.tile([N, w], F32, tag=f"bnc{jc}",
                                       name=f"bnc{jc}"))

            for jc in range(NJ):
                chains = [psp.tile([128, CW], F32, tag="chain",
                                   name=f"ch{jc}_{m}") for m in range(MB)]
                for k in range(KT):
                    for m in range(MB):
                        nc.tensor.matmul(
                            chains[m][:],
                            lhsT=ft_sb[k][:, m * 128:(m + 1) * 128],
                            rhs=ft_sb[k][:, jc * CW:(jc + 1) * CW],
                            start=(k == 0),
                            stop=(k == KT - 1),
                        )
                for m in range(MB):
                    g = gcp.tile([128, CW], F32, tag="g", name=f"g{jc}_{m}")
                    nc.vector.tensor_copy(g[:], chains[m][:])
                    nc.sync.dma_start(
                        bounce[jc][m * 128:(m + 1) * 128, 0:CW], g[:])
                    if m // 2 == jc:
                        # this chain holds diag block b=m at local cols
                        # 128*(m%2) .. +128
                        q = m % 2
                        junk = jkp.tile([128, CW], F32, tag="jk",
                                        name=f"jk{jc}_{m}")
                        nc.vector.tensor_tensor(
                            junk[:], chains[m][:], dm_sb[:, q, :], ALU.mult)
                        nc.vector.reduce_sum(sqp[:, m:m + 1], junk[:],
                                             axis=mybir.AxisListType.X)

            # ---- pack sq partials as 8 extra cols of last bounce ----
            for cp in range(MB):
                nc.sync.dma_start(
                    bounce[NJ - 1][cp * 128:(cp + 1) * 128, CW:CW + 8],
                    sqp[:])

            # ---- chunked ReduceScatter of the partial Gram ----
            rs = []
            for jc in range(NJ):
                w = CW + 8 if jc == NJ - 1 else CW
                r = drp.tile([128, w], F32, tag=f"rs{jc}", name=f"rs{jc}")
                nc.gpsimd.collective_compute(
                    "ReduceScatter",
                    ALU.add,
                    replica_groups=rg,
                    ins=[bounce[jc].opt()],
                    outs=[r.opt()],
                )
                rs.append(r)

            # ---- epilogue ----
            G_sb = misc.tile([128, N], F32, tag="G", name="G")
            for jc in range(NJ):
                nc.sync.dma_start(G_sb[:, jc * CW:(jc + 1) * CW],
                                  rs[jc][:, 0:CW])
            S_sb = misc.tile([128, 8], F32, tag="S", name="S")
            nc.sync.dma_start(S_sb[:], rs[NJ - 1][:, CW:CW + 8])
            flat = misc.tile([1, N], F32, tag="flat", name="flat")
            for b in range(MB):
                nc.sync.dma_start(flat[0:1, b * 128:(b + 1) * 128],
                                  rs[NJ - 1][:, CW + b:CW + b + 1])

            ones = misc.tile([1, 128], F32, tag="ones", name="ones")
            nc.vector.memset(ones[:], 1.0)

            # sq_row[i] = sq[128*core + i] via emask selection
            sq_row = misc.tile([128, 1], F32, tag="sqr", name="sqr")
            junk8 = misc.tile([128, 8], F32, tag="jk8", name="junk8")
            nc.vector.tensor_tensor(junk8[:], S_sb[:], em_sb[:], ALU.mult)
            nc.vector.reduce_sum(sq_row[:], junk8[:],
                                 axis=mybir.AxisListType.X)

            acc_h = []
            acc_e = []
            for h in range(2):
                # B = broadcast of sq over all partitions (fp32 matmul)
                Bh = psp.tile([128, 512], F32, tag="chain", name=f"B{h}")
                nc.tensor.matmul(Bh[:], lhsT=ones[:],
                                 rhs=flat[0:1, h * 512:(h + 1) * 512],
                                 start=True, stop=True)
                sl = slice(h * 512, (h + 1) * 512)
                t0 = jkp.tile([128, 512], F32, tag="t0", name=f"t0_{h}")
                nc.vector.scalar_tensor_tensor(
                    out=t0[:], in0=G_sb[:, sl], scalar=-2.0, in1=Bh[:],
                    op0=ALU.mult, op1=ALU.add)
                d = jkp.tile([128, 512], F32, tag="d", name=f"d{h}")
                nc.vector.tensor_scalar(
                    d[:], t0[:], sq_row[:], 0.0, ALU.add, ALU.max)
                ah = misc.tile([128, 1], F32, tag=f"ah{h}", name=f"ah{h}")
                jh = jkp.tile([128, 512], F32, tag="jh", name=f"jh{h}")
                nc.vector.tensor_tensor(jh[:], d[:], ms_sb[:, sl], ALU.mult)
                nc.vector.reduce_sum(ah[:], jh[:], axis=mybir.AxisListType.X)
                acc_h.append(ah)
                # min(d-1, 0) = -relu(1-d); heter partial = -sum(mask * that)
                # (negation applied on the host)
                t1 = jkp.tile([128, 512], F32, tag="t1", name=f"t1_{h}")
                nc.vector.tensor_scalar(
                    t1[:], d[:], -1.0, 0.0, ALU.add, ALU.min)
                eh = misc.tile([128, 1], F32, tag=f"eh{h}", name=f"eh{h}")
                je = jkp.tile([128, 512], F32, tag="je", name=f"je{h}")
                nc.vector.tensor_tensor(je[:], t1[:], md_sb[:, sl], ALU.mult)
                nc.vector.reduce_sum(eh[:], je[:], axis=mybir.AxisListType.X)
                acc_e.append(eh)

            out_sb = misc.tile([128, 2], F32, tag="osb", name="osb")
            nc.vector.tensor_tensor(out_sb[:, 0:1], acc_h[0][:], acc_h[1][:],
                                    ALU.add)
            nc.vector.tensor_tensor(out_sb[:, 1:2], acc_e[0][:], acc_e[1][:],
                                    ALU.add)
            nc.sync.dma_start(out[:], out_sb[:])

    nc.compile()
    return nc


def _host_inputs(x: np.ndarray):
    """Shard + transpose + cast x into per-core input maps."""
    f = np.ascontiguousarray(x.reshape(N, K))
    groups = np.arange(N) // BK
    cols = np.arange(N)

    dm = np.zeros((128, 2, CW), dtype=np.float32)
    for q in range(2):
        dm[np.arange(128), q, q * 128 + np.arange(128)] = 1.0

    in_maps = []
    for c in range(N_CORES):
        ftc = np.ascontiguousarray(
            f[:, c * KC:(c + 1) * KC].T).astype(ml_dtypes.bfloat16)
        rows = c * 128 + np.arange(128)
        g_r = groups[rows]
        same = ((g_r[:, None] == groups[None, :]) &
                (rows[:, None] != cols[None, :])).astype(np.float32)
        diff = (g_r[:, None] != groups[None, :]).astype(np.float32)
        em = np.zeros((128, 8), dtype=np.float32)
        em[:, c] = 1.0
        in_maps.append({
            "ft": ftc.reshape(KT, 128, N),
            "mask_same": same,
            "mask_diff": diff,
            "diagmask": dm,
            "emask": em,
        })
    return in_maps


def kernel(x: np.ndarray):
    if "nc" not in _CACHE:
        _CACHE["nc"] = _build_nc()
    nc = _CACHE["nc"]

    in_maps = _host_inputs(x)
    res = bass_utils.run_bass_kernel_spmd(
        nc, in_maps, core_ids=list(range(N_CORES)))

    total_h = 0.0
    total_e = 0.0
    for c in range(N_CORES):
        o = res.results[c]["out"].astype(np.float64)
        total_h += o[:, 0].sum()
        total_e += o[:, 1].sum()

    # reference: 2 * (0.5 * sum_same dist) / (N * (BK - 1))
    #            2 * sum_diff relu(1 - dist) / (N * (N // BK - 1))
    # device accumulates sum(min(dist-1, 0) * mask_diff) = -heter partial
    homo = total_h / (N * (BK - 1))
    heter = -2.0 * total_e / (N * (N // BK - 1))
    return (np.float32(homo), np.float32(heter))
